# revision 53
# baseline (speedup 1.0000x reference)
"""MultiHeadDiffAttention Trainium2 kernel (8 NeuronCores).

Sharding: batch (4) x head-group (2 groups of 8 heads) = 8 cores.
Each core computes a partial (T, C) c_proj output for its batch element
restricted to its 8 heads; the host sums the two head-group f16 partials
per batch element in f32.

Per-core pipeline (all matmuls on PE in fp16, PSUM accumulation fp32):
  1. Host pre-transposes x[b] -> xT (C on partitions) and interleaves the
     Q/K weights so each 128-col chunk is one head's [q1|q2] dims. The
     per-head V blocks of Wv are column-mean-centered on the host, which
     makes y = att @ V exactly row-mean-free: the LayerNorm mean
     subtraction vanishes (mu == 0) and the LN apply is a pure scale.
  2. DMA priority xT -> wv -> wc on the sync/scalar queue pair (the
     startup is HBM-bandwidth-bound); dummy matmuls keep the PE HAM
     clock at 8/8 until xT lands.
  3. Per head/stream: scores S^T(k,q) = K^T-tiles x Q^T (contract d=64),
     exp via ScalarE with fused scale 1/8 and bias -6*ln2 (the 2^-6
     keeps the unnormalized y in f16 range; LN scale-invariance makes it
     exact) -> fp16 P, causal mask via gpsimd affine_select, then PV:
     P-tile^T x [V|1] accumulated over k-tiles gives Y and the softmax
     denominator in one matmul. PV of head h-1 (and the V projection,
     for h=0) is interleaved between head h's score units; head 7's
     first four q-tiles' PV rides inside head 7's own iteration.
  4. Streams combined as z = Y1 - (lam*den1/den2) * Y2 (per-q scalars),
     which equals den1 * (a1 - lam*a2) @ V; LayerNorm is scale-invariant
     per row, so normalizing z with eps scaled by den1^2 reproduces the
     reference exactly.
  5. LN invstd via bn_stats/bn_aggr + exp(-0.5*ln(var+eps*den1^2) +
     ln(1-li)); the apply (a row scale) is split across DVE and ACT.
     LN chains are emitted just-in-time so the DVE FIFO never delays
     the tail PV psum drains (PE stalls via psum WAR otherwise, and any
     >~25% idle HAM window re-throttles the PE clock to 1.2 GHz).
  6. f16 PE-transpose of y_ln (f16 PSUM), c_proj vs host-sliced f16 Wc
     rows -> f16 partial out, with tile i+1's transposes pipelined under
     tile i's c_proj and the final tile in 256-col groups to shorten the
     end-of-kernel chain.
"""

import contextlib
import ctypes
import math
import sys
import types

import numpy as np

sys.path.insert(0, "/opt/trn_rl_repo")


def _install_ntff_hook():
    """Provide antenv.axon_hooks if the image lacks it (for trace=True)."""
    try:
        from antenv.axon_hooks import get_axon_ntff_profile_hook  # noqa: F401

        return
    except ImportError:
        pass

    so_path = "/opt/axon/libaxon_pjrt.so"

    def _make_hook():
        try:
            lib = ctypes.CDLL(so_path)
        except OSError:
            return None
        if not hasattr(lib, "axon_start_nrt_profile"):
            return None
        lib.axon_start_nrt_profile.argtypes = [
            ctypes.POINTER(ctypes.c_int64),
            ctypes.c_size_t,
        ]
        lib.axon_start_nrt_profile.restype = ctypes.c_int64
        lib.axon_stop_nrt_profile.argtypes = [ctypes.c_char_p]
        lib.axon_stop_nrt_profile.restype = ctypes.c_int64

        @contextlib.contextmanager
        def _hook(output_dir, device_ids):
            import jax

            jax.devices()
            if device_ids:
                ids = (ctypes.c_int64 * len(device_ids))(*device_ids)
                rc = lib.axon_start_nrt_profile(ids, len(device_ids))
            else:
                rc = lib.axon_start_nrt_profile(None, 0)
            if rc != 0:
                raise RuntimeError(f"axon_start_nrt_profile rc={rc}")
            try:
                yield
            finally:
                n = lib.axon_stop_nrt_profile(str(output_dir).encode())
                if n < 0:
                    raise RuntimeError(f"axon_stop_nrt_profile rc={n}")

        return _hook

    mod = types.ModuleType("antenv.axon_hooks")
    _the_hook = _make_hook()
    mod.get_axon_ntff_profile_hook = lambda: _the_hook
    sys.modules["antenv.axon_hooks"] = mod


_install_ntff_hook()

import concourse.bass as bass  # noqa: E402
import concourse.bass_utils as bass_utils_mod  # noqa: E402
import concourse.mybir as mybir  # noqa: E402
import concourse.tile as tile  # noqa: E402
from concourse.masks import make_identity  # noqa: E402


def _enable_ldw_opt():
    """walrus ships with --enable-ldw-opt=false hardcoded; the LDWEIGHTS
    optimization pass overlaps weight loads with in-flight matmuls, which
    matters a lot for this kernel (a fresh stationary operand per matmul
    in the PV stage). Rewrite the flag on the walrus command line."""
    orig = bass_utils_mod.run_command

    def patched(argv, **kwargs):
        argv = [
            "--enable-ldw-opt=true" if a == "--enable-ldw-opt=false" else a
            for a in argv
        ]
        return orig(argv, **kwargs)

    bass_utils_mod.run_command = patched


import os  # noqa: E402

if os.environ.get("BASS_LDW_OPT", "0") == "1":
    _enable_ldw_opt()

P = 128
T = 1024
C = 1024
NH = 8  # heads per core
HS = 64
LAMBDA_INIT = 0.8 - 0.6 * math.exp(-0.3 * (2 - 1))
LN_EPS = 1e-5
N_CORES = 8

f32 = mybir.dt.float32
f32r = mybir.dt.float32r
f16 = mybir.dt.float16
Alu = mybir.AluOpType
Act = mybir.ActivationFunctionType


def r(ap):
    return ap.bitcast(f32r)


def build_program():
    nc = bass.Bass()
    xt_d = nc.dram_tensor("xt", [C, T], f16, kind="ExternalInput")
    wq_d = nc.dram_tensor("wq", [C, C], f16, kind="ExternalInput")
    wk_d = nc.dram_tensor("wk", [C, C], f16, kind="ExternalInput")
    wv_d = nc.dram_tensor("wv", [C, C], f16, kind="ExternalInput")
    wc_d = nc.dram_tensor("wc", [C, C], f16, kind="ExternalInput")
    lamneg_d = nc.dram_tensor("lamneg", [P, NH], f32, kind="ExternalInput")
    out_d = nc.dram_tensor("out", [T, C], f16, kind="ExternalOutput")

    ln_bias = float(math.log(1.0 - LAMBDA_INIT))
    # fold 2^-6 into the exp so the unnormalized y fits comfortably in f16;
    # the LN scale-invariance (eps scaled by den1^2) makes this exact
    exp_bias = float(-6.0 * math.log(2.0))

    with tile.TileContext(nc) as tc:
        with (
            tc.tile_pool(name="const", bufs=1) as const,
            tc.tile_pool(name="ydata", bufs=8) as y_pool,
            tc.tile_pool(name="vdata", bufs=8) as v_p,
            tc.tile_pool(name="wcp", bufs=8) as wc_p,
        ):
            wc_sb = [wc_p.tile([P, C], f16, tag="wc", name="wcsb") for _ in range(8)]
            dum = const.tile([P, 512], f16, tag="dum")
            nc.gpsimd.memset(dum, 0.0)
            ident = const.tile([P, P], f16, tag="ident")
            make_identity(nc, ident)
            lamneg = const.tile([P, NH], f32, tag="lamneg")
            den_store = const.tile([P, NH, 8], f32, tag="den")
            lnb = const.tile([P, 1], f32, tag="lnb")
            nc.vector.memset(lnb, ln_bias)
            expb = const.tile([P, 1], f32, tag="expb")
            nc.vector.memset(expb, exp_bias)

            y_tiles = [y_pool.tile([P, NH * P], f16, tag="y", name="yt") for _ in range(8)]
            var_tiles = [y_pool.tile([P, NH], f32, tag="var", name="var") for _ in range(8)]
            v_aug = [v_p.tile([P, NH, 132], f16, tag="v", name="vaug") for _ in range(8)]

            p_ctx = tc.tile_pool(name="pprob", bufs=4)
            p_pool = p_ctx.__enter__()
            small_ctx = tc.tile_pool(name="smallc", bufs=16)
            small = small_ctx.__enter__()

            def pv_unit(h, s, i, pcs, ypool, tail=False):
                """PV + stream-combine + LN stats for one q-tile.

                tail=True routes the PSUM drains to the scalar engine (idle
                there) so the DVE never back-pressures the PE via psum WAR.
                """
                n, t = i // 4, i % 4
                pch = pcs[(s, n)]
                yp = ypool.tile([P, 129], f32, tag="psY", name="yp")
                for j in range(i + 1):
                    nc.tensor.matmul(
                        yp,
                        lhsT=pch[:, j, 128 * t : 128 * (t + 1)],
                        rhs=v_aug[j][:, h, 0:129],
                        start=(j == 0),
                        stop=(j == i),
                    )
                ysl = y_tiles[i][:, 128 * h : 128 * (h + 1)]
                if s == 0:
                    if tail:
                        nc.scalar.activation(out=ysl, in_=yp[:, 0:128], func=Act.Copy)
                    else:
                        nc.vector.tensor_copy(out=ysl, in_=yp[:, 0:128])
                    nc.vector.tensor_copy(
                        out=den_store[:, h, i : i + 1], in_=yp[:, 128:129]
                    )
                else:
                    r2 = small.tile([P, 1], f32, tag="r2", name="r2")
                    nc.vector.reciprocal(out=r2, in_=yp[:, 128:129])
                    gneg = small.tile([P, 1], f32, tag="gneg", name="gneg")
                    nc.vector.tensor_mul(
                        out=gneg, in0=den_store[:, h, i : i + 1], in1=r2
                    )
                    nc.vector.tensor_mul(
                        out=gneg, in0=gneg, in1=lamneg[:, h : h + 1]
                    )
                    tmp = small.tile([P, P], f16, tag="tmp", name="tmp")
                    if tail:
                        nc.scalar.activation(
                            out=tmp, in_=yp[:, 0:128], func=Act.Copy, scale=gneg
                        )
                    else:
                        nc.vector.tensor_scalar(
                            out=tmp, in0=yp[:, 0:128], scalar1=gneg, scalar2=None,
                            op0=Alu.mult,
                        )
                    nc.vector.tensor_add(out=ysl, in0=ysl, in1=tmp)
                    bs = small.tile(
                        [P, nc.vector.BN_STATS_DIM], f32, tag="bs", name="bs"
                    )
                    nc.vector.bn_stats(out=bs, in_=ysl)
                    mv = small.tile(
                        [P, nc.vector.BN_AGGR_DIM], f32, tag="mv", name="mv"
                    )
                    nc.vector.bn_aggr(out=mv, in_=bs)
                    nc.vector.tensor_copy(
                        out=var_tiles[i][:, h : h + 1], in_=mv[:, 1:2]
                    )


            def emit_ln(i):
                # veps = var + eps*den1^2 -> invstd via Ln/Exp (both live
                # in the natural_log_exp ACT table set: one switch total)
                d1 = den_store[:, :, i : i + 1].rearrange("p h one -> p (h one)")
                veps = small.tile([P, NH], f32, tag="veps", name="veps")
                nc.vector.tensor_mul(out=veps, in0=d1, in1=d1)
                nc.vector.tensor_scalar(
                    out=veps, in0=veps, scalar1=LN_EPS, scalar2=None,
                    op0=Alu.mult,
                )
                nc.vector.tensor_add(out=veps, in0=veps, in1=var_tiles[i])
                inv = small.tile([P, NH], f32, tag="inv", name="inv")
                nc.scalar.activation(out=inv, in_=veps, func=Act.Ln)
                nc.scalar.activation(
                    out=inv, in_=inv, func=Act.Exp, scale=-0.5, bias=lnb
                )
                # V was host-centered, so mu == 0 and the LN apply is a
                # pure per-row scale; split it across DVE and ACT
                for h in range(NH):
                    ysl = y_tiles[i][:, 128 * h : 128 * (h + 1)]
                    if h % 2 == 0:
                        nc.vector.tensor_scalar(
                            out=ysl, in0=ysl,
                            scalar1=inv[:, h : h + 1], scalar2=None,
                            op0=Alu.mult,
                        )
                    else:
                        nc.scalar.activation(
                            out=ysl, in_=ysl, func=Act.Copy,
                            scale=inv[:, h : h + 1],
                        )

            # ---------- merged projections + attention ----------
            with (
                tc.tile_pool(name="xT", bufs=8) as xT_p,
                tc.tile_pool(name="wv", bufs=8) as wv_p,
                tc.tile_pool(name="wqk", bufs=2) as wqk_p,
                tc.tile_pool(name="qk", bufs=2) as qk_p,
                tc.tile_pool(name="psB2", bufs=2, space="PSUM") as psB2,
                tc.tile_pool(name="psS", bufs=4, space="PSUM") as psS,
                tc.tile_pool(name="psY", bufs=2, space="PSUM") as psY,
            ):
                xT = [xT_p.tile([P, T], f16, tag="xT", name="xT") for _ in range(8)]
                wv_sb = [wv_p.tile([P, C], f16, tag="w", name="wsb") for _ in range(8)]

                def emit_wdma(h, eng=None):
                    """One strided DMA per weight matrix for head h:
                    out[p, c, d] = w[128c + p, 128h + d]."""
                    tiles = []
                    for w_d, tag, nm in ((wq_d, "wq", "wqh"), (wk_d, "wk", "wkh")):
                        wt = wqk_p.tile([P, 8, P], f16, tag=tag, name=nm)
                        src_ap = w_d.rearrange("(c p) d -> p c d", p=P)[
                            :, :, 128 * h : 128 * (h + 1)
                        ]
                        (eng or nc.sync).dma_start(out=wt, in_=src_ap)
                        tiles.append(wt)
                    return tiles

                def emit_proj(wt, dest):
                    """(head_dim 128, T) projection for one head."""
                    for n in range(2):
                        ps = psB2.tile([P, 512], f32, tag="psB2", name="pps")
                        for c in range(8):
                            nc.tensor.matmul(
                                ps,
                                lhsT=wt[:, c, :],
                                rhs=xT[c][:, 512 * n : 512 * (n + 1)],
                                start=(c == 0),
                                stop=(c == 7),
                            )
                        nc.vector.tensor_copy(
                            out=dest[:, 512 * n : 512 * (n + 1)], in_=ps
                        )

                def score_unit(h, qT, kT, pcs, n, j):
                    """One k-tile of scores: both streams' matmuls in
                    concurrent PE row-groups, then exp + diag mask."""
                    qlo = 128 * max(0, j - 4 * n)
                    sp2 = [
                        psS.tile([P, 512], f32, tag="psS", name="sp")
                        for _ in range(2)
                    ]
                    for s in range(2):
                        nc.tensor.matmul(
                            sp2[s][:, qlo:512],
                            lhsT=kT[64 * s : 64 * (s + 1), 128 * j : 128 * (j + 1)],
                            rhs=qT[
                                64 * s : 64 * (s + 1),
                                512 * n + qlo : 512 * (n + 1),
                            ],
                            start=True,
                            stop=True,
                        )
                    t = j - 4 * n
                    for s in range(2):
                        pch = pcs[(s, n)]
                        nc.scalar.activation(
                            out=pch[:, j, qlo:512],
                            in_=sp2[s][:, qlo:512],
                            func=Act.Exp,
                            scale=0.125,
                            bias=expb,
                        )
                        if 0 <= t <= 3:
                            nc.gpsimd.affine_select(
                                out=pch[:, j, 128 * t : 128 * (t + 1)],
                                in_=pch[:, j, 128 * t : 128 * (t + 1)],
                                compare_op=Alu.is_ge,
                                fill=0.0,
                                base=0,
                                pattern=[[1, 128]],
                                channel_multiplier=-1,
                            )

                def vproj_unit(t):
                    """V-projection tile, both 512-col halves."""
                    for n in range(2):
                        ps = psB2.tile([P, 512], f32, tag="psB2", name="pps")
                        for c in range(8):
                            nc.tensor.matmul(
                                ps,
                                lhsT=xT[c][:, 128 * t : 128 * (t + 1)],
                                rhs=wv_sb[c][:, 512 * n : 512 * (n + 1)],
                                start=(c == 0),
                                stop=(c == 7),
                            )
                        nc.any.tensor_copy(
                            out=v_aug[t][:, 4 * n : 4 * (n + 1), 0:128],
                            in_=ps.rearrange("p (g d) -> p g d", g=4),
                        )
                    nc.vector.memset(v_aug[t][:, :, 128:129], 1.0)

                # software pipeline: PV matmuls of head h-1 (and the
                # V-projection, for h=0) are interleaved between head h's
                # score units so PE never stalls on exp
                # PE warm-up: dummy matmuls with no input deps keep the
                # HAM clock at 8/8 while the initial DMAs land
                def emit_dummies(k):
                    for _ in range(k):
                        scr = psS.tile([P, 512], f32, tag="psS", name="sp")
                        for _ in range(8):
                            nc.tensor.matmul(
                                scr, lhsT=dum[:, 0:128], rhs=dum,
                                start=True, stop=True,
                            )

                emit_dummies(2)

                # critical path first: head-0 weights + xT split over the
                # sync/scalar queues; wv owns the gpsimd queue (needed later)
                wt0 = wqk_p.tile([P, 8, P], f16, tag="wq", name="wqh")
                nc.sync.dma_start(
                    out=wt0,
                    in_=wq_d.rearrange("(c p) d -> p c d", p=P)[:, :, 0:128],
                )
                wt1 = wqk_p.tile([P, 8, P], f16, tag="wk", name="wkh")
                nc.scalar.dma_start(
                    out=wt1,
                    in_=wk_d.rearrange("(c p) d -> p c d", p=P)[:, :, 0:128],
                )
                wts = [wt0, wt1]
                # DMA priority: xT first (gates head-0 projections), then wv
                # (needed ~15us in, during head-0's score phase), then wc
                # (needed only at the tail) — all on the sync/scalar queue
                # pair so the later loads cannot steal HBM bandwidth from xT
                for c in range(8):
                    eng = nc.sync if c % 2 == 0 else nc.scalar
                    eng.dma_start(out=xT[c], in_=xt_d[128 * c : 128 * (c + 1), :])
                for c in range(8):
                    eng = nc.sync if c % 2 == 0 else nc.scalar
                    eng.dma_start(out=wv_sb[c], in_=wv_d[128 * c : 128 * (c + 1), :])
                for dd in range(8):
                    eng = nc.sync if dd % 2 == 0 else nc.scalar
                    eng.dma_start(out=wc_sb[dd], in_=wc_d[128 * dd : 128 * (dd + 1), :])
                nc.gpsimd.dma_start(out=lamneg, in_=lamneg_d[:, :])
                pcs_prev = None
                for h in range(NH):
                    qT = qk_p.tile([P, T], f16, tag="q", name="qT")
                    kT = qk_p.tile([P, T], f16, tag="k", name="kT")
                    if h == 0:
                        # head 0 is paced by the xT DMA: interleave Q and K
                        # per c-chunk so each chunk is consumed as it lands
                        # (K's accumulators borrow psS - scores idle so far)
                        psq = [
                            psB2.tile([P, 512], f32, tag="psB2", name="pq")
                            for _ in range(2)
                        ]
                        psk = [
                            psS.tile([P, 512], f32, tag="psS", name="pk")
                            for _ in range(2)
                        ]
                        for c in range(8):
                            for n in range(2):
                                nc.tensor.matmul(
                                    psq[n],
                                    lhsT=wts[0][:, c, :],
                                    rhs=xT[c][:, 512 * n : 512 * (n + 1)],
                                    start=(c == 0),
                                    stop=(c == 7),
                                )
                            for n in range(2):
                                nc.tensor.matmul(
                                    psk[n],
                                    lhsT=wts[1][:, c, :],
                                    rhs=xT[c][:, 512 * n : 512 * (n + 1)],
                                    start=(c == 0),
                                    stop=(c == 7),
                                )
                        for n in range(2):
                            nc.vector.tensor_copy(
                                out=qT[:, 512 * n : 512 * (n + 1)], in_=psq[n]
                            )
                            nc.vector.tensor_copy(
                                out=kT[:, 512 * n : 512 * (n + 1)], in_=psk[n]
                            )
                    else:
                        emit_proj(wts[0], qT)
                        emit_proj(wts[1], kT)
                    if h + 1 < NH:
                        next_wts = emit_wdma(h + 1)
                    pcs = {
                        (s, n): p_pool.tile(
                            [P, 4 * n + 4, 512], f16,
                            tag=f"p{n}", name="pch", bufs=4,
                        )
                        for s in range(2)
                        for n in range(2)
                    }
                    if h == 0:
                        backlog = [("v", t) for t in range(8)]
                    else:
                        backlog = [("pv", s, i) for i in range(8) for s in range(2)]
                    if h == NH - 1:
                        # head 7 has no successor iteration: pull its first
                        # four q-tiles' PV units into this head's backlog,
                        # spread through the drain order as their exp chunks
                        # become available
                        for pos, ti in ((8, 3), (12, 2), (16, 1), (20, 0)):
                            backlog[pos:pos] = [("pv7", 0, ti), ("pv7", 1, ti)]
                    sunits = [(n, j) for n in range(2) for j in range(4 * n + 4)]
                    done = 0
                    for idx, (n, j) in enumerate(sunits):
                        score_unit(h, qT, kT, pcs, n, j)
                        while done < len(backlog) and (idx + 1) * len(
                            backlog
                        ) >= (done + 1) * len(sunits):
                            u = backlog[done]
                            done += 1
                            # late head-7 backlog units drain on ACT: exp is
                            # finished there, and the DVE otherwise paces the
                            # PE via psum WAR with no score units left to fill
                            late = h == NH - 1 and done > 14
                            if u[0] == "v":
                                vproj_unit(u[1])
                            elif u[0] == "pv7":
                                pv_unit(h, u[1], u[2], pcs, psY, tail=late)
                            else:
                                pv_unit(h - 1, u[1], u[2], pcs_prev, psY, tail=late)
                    while done < len(backlog):
                        u = backlog[done]
                        done += 1
                        late = h == NH - 1 and done > 14
                        if u[0] == "v":
                            vproj_unit(u[1])
                        elif u[0] == "pv7":
                            pv_unit(h, u[1], u[2], pcs, psY, tail=late)
                        else:
                            pv_unit(h - 1, u[1], u[2], pcs_prev, psY, tail=late)
                    if h == NH - 1:
                        # tiles 3 and 2 finished PV above: run their LN now
                        # so the tail's first transposes never wait on it
                        emit_ln(3)
                        emit_ln(2)
                        # filler matmuls bridge the pool-transition stall so
                        # the HAM activity window never sees the PE idle
                        emit_dummies(1)
                    pcs_prev = pcs
                    if h + 1 < NH:
                        wts = next_wts

            # ---------- tail: PV(7) + LN + transpose pipeline, then c_proj ----------
            with (
                tc.tile_pool(name="smalld", bufs=10) as sd,
                tc.tile_pool(name="ylnT", bufs=8) as ylnT_p,
                tc.tile_pool(name="outp", bufs=3) as out_p,
                tc.tile_pool(name="psY2", bufs=3, space="PSUM") as psY2,
                tc.tile_pool(name="psE", bufs=3, space="PSUM") as psE,
                tc.tile_pool(name="psF", bufs=2, space="PSUM") as psF,
            ):
                ylnT = [ylnT_p.tile([P, T], f16, tag="ylnT", name="ylnT") for _ in range(8)]

                def emit_trans(i):
                    for dh in range(2):
                        pt = psE.tile([P, 512], f16, tag="psE")
                        for w in range(4):
                            dd = 4 * dh + w
                            nc.tensor.transpose(
                                out=pt[:, 128 * w : 128 * (w + 1)],
                                in_=y_tiles[i][:, 128 * dd : 128 * (dd + 1)],
                                identity=ident,
                            )
                        for w in range(4):
                            dd = 4 * dh + w
                            nc.any.tensor_copy(
                                out=ylnT[dd][:, 128 * i : 128 * (i + 1)],
                                in_=pt[:, 128 * w : 128 * (w + 1)],
                            )

                def emit_cproj(i, last=False):
                    # c_proj row-block m = i only needs this q-tile's
                    # transposed columns; the final tile uses 256-col groups
                    # to shorten the end-of-kernel dependency chain
                    osb = out_p.tile([P, C], f16, tag="osb")
                    ngrp, w = (4, 256) if last else (2, 512)
                    for n in range(ngrp):
                        ps = psF.tile([P, w], f32, tag="psF", name="psn")
                        for d in range(8):
                            nc.tensor.matmul(
                                ps,
                                lhsT=ylnT[d][:, 128 * i : 128 * (i + 1)],
                                rhs=wc_sb[d][:, w * n : w * (n + 1)],
                                start=(d == 0),
                                stop=(d == 7),
                            )
                        if last and n % 2 == 1:
                            nc.scalar.activation(
                                out=osb[:, w * n : w * (n + 1)],
                                in_=ps, func=Act.Copy,
                            )
                        else:
                            nc.any.tensor_copy(
                                out=osb[:, w * n : w * (n + 1)], in_=ps
                            )
                        nc.sync.dma_start(
                            out=out_d[128 * i : 128 * (i + 1), w * n : w * (n + 1)],
                            in_=osb[:, w * n : w * (n + 1)],
                        )


                # q-tiles 0-3 finished PV inside the head-7 loop: emit their
                # LN chains first (pure DVE/ACT), then interleave the
                # remaining PV units with their trans+cproj so the PE never
                # waits on the LN latency
                # LN chains are emitted just-in-time (two per iteration) so
                # the DVE queue never delays the tail PV psum drains, which
                # would stall the PE via psum WAR
                ready = [3, 2, 1, 0]
                jit = [1, 0, None, None]
                for k, i in enumerate([7, 6, 5, 4]):
                    for s in range(2):
                        pv_unit(NH - 1, s, i, pcs_prev, psY2)
                    # trans+cproj first: its DVE copies must not queue behind
                    # the LN chains in the vector engine's FIFO
                    emit_trans(ready[k])
                    emit_cproj(ready[k])
                    emit_ln(i)
                    if jit[k] is not None:
                        emit_ln(jit[k])
                # software-pipeline the final four tiles: tile i+1's
                # transposes (and their copies) run under tile i's c_proj
                emit_trans(7)
                emit_trans(6)
                emit_cproj(7)
                emit_trans(5)
                emit_cproj(6)
                emit_trans(4)
                emit_cproj(5)
                emit_cproj(4, last=True)

            small_ctx.__exit__(None, None, None)
            p_ctx.__exit__(None, None, None)

    bass._bass_rust.generate_event_semaphores(nc)
    return nc


_NC = None


def _get_program():
    global _NC
    if _NC is None:
        _NC = build_program()
    return _NC


def make_in_maps(inputs):
    """Host-side sharding: per-core input dicts."""
    x = np.ascontiguousarray(np.asarray(inputs["x"], dtype=np.float32))
    Wq1 = np.asarray(inputs["Wq1"], dtype=np.float32)
    Wq2 = np.asarray(inputs["Wq2"], dtype=np.float32)
    Wk1 = np.asarray(inputs["Wk1"], dtype=np.float32)
    Wk2 = np.asarray(inputs["Wk2"], dtype=np.float32)
    Wv = np.asarray(inputs["Wv"], dtype=np.float32)
    Wc = np.asarray(inputs["Wc"], dtype=np.float32)
    lq1 = np.asarray(inputs["lq1"], dtype=np.float32)
    lk1 = np.asarray(inputs["lk1"], dtype=np.float32)
    lq2 = np.asarray(inputs["lq2"], dtype=np.float32)
    lk2 = np.asarray(inputs["lk2"], dtype=np.float32)

    lam1 = np.exp(np.sum(lq1 * lk1, axis=-1))
    lam2 = np.exp(np.sum(lq2 * lk2, axis=-1))
    lam_full = (lam1 - lam2 + LAMBDA_INIT).astype(np.float32)  # (16,)

    in_maps = []
    for core in range(N_CORES):
        b, hg = core // 2, core % 2
        heads = np.arange(NH) + NH * hg  # global head idx
        wq = np.empty((C, C), np.float32)
        wk = np.empty((C, C), np.float32)
        wv = np.empty((C, C), np.float32)
        for h in range(NH):
            H = NH * hg + h
            wq[:, 128 * h : 128 * h + 64] = Wq1[:, HS * H : HS * (H + 1)]
            wq[:, 128 * h + 64 : 128 * (h + 1)] = Wq2[:, HS * H : HS * (H + 1)]
            wk[:, 128 * h : 128 * h + 64] = Wk1[:, HS * H : HS * (H + 1)]
            wk[:, 128 * h + 64 : 128 * (h + 1)] = Wk2[:, HS * H : HS * (H + 1)]
            # center each head's V block: y = att @ V is then exactly
            # row-mean-free, so the LN mean subtraction vanishes (mu == 0)
            blk = Wv[:, 128 * H : 128 * (H + 1)]
            wv[:, 128 * h : 128 * (h + 1)] = blk - blk.mean(axis=1, keepdims=True)
        wc = np.ascontiguousarray(Wc[1024 * hg : 1024 * (hg + 1), :]).astype(np.float16)
        lamneg = np.broadcast_to(
            -lam_full[heads][None, :], (P, NH)
        ).astype(np.float32)
        in_maps.append(
            {
                "xt": np.ascontiguousarray(x[b].T.astype(np.float16)),
                "wq": wq.astype(np.float16),
                "wk": wk.astype(np.float16),
                "wv": wv.astype(np.float16),
                "wc": wc,
                "lamneg": np.ascontiguousarray(lamneg),
            }
        )
    return in_maps


def run(inputs, trace=False, **kw):
    from concourse.bass_utils import run_bass_kernel_spmd

    nc = _get_program()
    in_maps = make_in_maps(inputs)
    res = run_bass_kernel_spmd(
        nc, in_maps, core_ids=list(range(N_CORES)), trace=trace, **kw
    )
    B = 4
    out = np.empty((B, T, C), np.float32)
    for b in range(B):
        out[b] = (
            res.results[2 * b]["out"].astype(np.float32)
            + res.results[2 * b + 1]["out"].astype(np.float32)
        )
    return out, res


def kernel(**inputs) -> np.ndarray:
    out, _ = run(inputs, trace=False)
    return out



# revision 54
# speedup vs baseline: 1.0004x; 1.0004x over previous
"""MultiHeadDiffAttention Trainium2 kernel (8 NeuronCores).

Sharding: batch (4) x head-group (2 groups of 8 heads) = 8 cores.
Each core computes a partial (T, C) c_proj output for its batch element
restricted to its 8 heads; the host sums the two head-group f16 partials
per batch element in f32.

Per-core pipeline (all matmuls on PE in fp16, PSUM accumulation fp32):
  1. Host pre-transposes x[b] -> xT (C on partitions) and interleaves the
     Q/K weights so each 128-col chunk is one head's [q1|q2] dims. The
     per-head V blocks of Wv are column-mean-centered on the host, which
     makes y = att @ V exactly row-mean-free: the LayerNorm mean
     subtraction vanishes (mu == 0) and the LN apply is a pure scale.
  2. DMA priority xT -> wv -> wc on the sync/scalar queue pair (the
     startup is HBM-bandwidth-bound); dummy matmuls keep the PE HAM
     clock at 8/8 until xT lands.
  3. Per head/stream: scores S^T(k,q) = K^T-tiles x Q^T (contract d=64),
     exp via ScalarE with fused scale 1/8 and bias -6*ln2 (the 2^-6
     keeps the unnormalized y in f16 range; LN scale-invariance makes it
     exact) -> fp16 P, causal mask via gpsimd affine_select, then PV:
     P-tile^T x [V|1] accumulated over k-tiles gives Y and the softmax
     denominator in one matmul. PV of head h-1 (and the V projection,
     for h=0) is interleaved between head h's score units; head 7's
     first four q-tiles' PV rides inside head 7's own iteration.
  4. Streams combined as z = Y1 - (lam*den1/den2) * Y2 (per-q scalars),
     which equals den1 * (a1 - lam*a2) @ V; LayerNorm is scale-invariant
     per row, so normalizing z with eps scaled by den1^2 reproduces the
     reference exactly.
  5. LN invstd via bn_stats/bn_aggr + exp(-0.5*ln(var+eps*den1^2) +
     ln(1-li)); the apply (a row scale) is split across DVE and ACT.
     LN chains are emitted just-in-time so the DVE FIFO never delays
     the tail PV psum drains (PE stalls via psum WAR otherwise, and any
     >~25% idle HAM window re-throttles the PE clock to 1.2 GHz).
  6. f16 PE-transpose of y_ln (f16 PSUM), c_proj vs host-sliced f16 Wc
     rows -> f16 partial out, with tile i+1's transposes pipelined under
     tile i's c_proj and the final tile in 256-col groups to shorten the
     end-of-kernel chain.
"""

import contextlib
import ctypes
import math
import sys
import types

import numpy as np

sys.path.insert(0, "/opt/trn_rl_repo")


def _install_ntff_hook():
    """Provide antenv.axon_hooks if the image lacks it (for trace=True)."""
    try:
        from antenv.axon_hooks import get_axon_ntff_profile_hook  # noqa: F401

        return
    except ImportError:
        pass

    so_path = "/opt/axon/libaxon_pjrt.so"

    def _make_hook():
        try:
            lib = ctypes.CDLL(so_path)
        except OSError:
            return None
        if not hasattr(lib, "axon_start_nrt_profile"):
            return None
        lib.axon_start_nrt_profile.argtypes = [
            ctypes.POINTER(ctypes.c_int64),
            ctypes.c_size_t,
        ]
        lib.axon_start_nrt_profile.restype = ctypes.c_int64
        lib.axon_stop_nrt_profile.argtypes = [ctypes.c_char_p]
        lib.axon_stop_nrt_profile.restype = ctypes.c_int64

        @contextlib.contextmanager
        def _hook(output_dir, device_ids):
            import jax

            jax.devices()
            if device_ids:
                ids = (ctypes.c_int64 * len(device_ids))(*device_ids)
                rc = lib.axon_start_nrt_profile(ids, len(device_ids))
            else:
                rc = lib.axon_start_nrt_profile(None, 0)
            if rc != 0:
                raise RuntimeError(f"axon_start_nrt_profile rc={rc}")
            try:
                yield
            finally:
                n = lib.axon_stop_nrt_profile(str(output_dir).encode())
                if n < 0:
                    raise RuntimeError(f"axon_stop_nrt_profile rc={n}")

        return _hook

    mod = types.ModuleType("antenv.axon_hooks")
    _the_hook = _make_hook()
    mod.get_axon_ntff_profile_hook = lambda: _the_hook
    sys.modules["antenv.axon_hooks"] = mod


_install_ntff_hook()

import concourse.bass as bass  # noqa: E402
import concourse.bass_utils as bass_utils_mod  # noqa: E402
import concourse.mybir as mybir  # noqa: E402
import concourse.tile as tile  # noqa: E402
from concourse.masks import make_identity  # noqa: E402


def _enable_ldw_opt():
    """walrus ships with --enable-ldw-opt=false hardcoded; the LDWEIGHTS
    optimization pass overlaps weight loads with in-flight matmuls, which
    matters a lot for this kernel (a fresh stationary operand per matmul
    in the PV stage). Rewrite the flag on the walrus command line."""
    orig = bass_utils_mod.run_command

    def patched(argv, **kwargs):
        argv = [
            "--enable-ldw-opt=true" if a == "--enable-ldw-opt=false" else a
            for a in argv
        ]
        return orig(argv, **kwargs)

    bass_utils_mod.run_command = patched


import os  # noqa: E402

if os.environ.get("BASS_LDW_OPT", "0") == "1":
    _enable_ldw_opt()

P = 128
T = 1024
C = 1024
NH = 8  # heads per core
HS = 64
LAMBDA_INIT = 0.8 - 0.6 * math.exp(-0.3 * (2 - 1))
LN_EPS = 1e-5
N_CORES = 8

f32 = mybir.dt.float32
f32r = mybir.dt.float32r
f16 = mybir.dt.float16
Alu = mybir.AluOpType
Act = mybir.ActivationFunctionType


def r(ap):
    return ap.bitcast(f32r)


def build_program():
    nc = bass.Bass()
    xt_d = nc.dram_tensor("xt", [C, T], f16, kind="ExternalInput")
    wq_d = nc.dram_tensor("wq", [C, C], f16, kind="ExternalInput")
    wk_d = nc.dram_tensor("wk", [C, C], f16, kind="ExternalInput")
    wv_d = nc.dram_tensor("wv", [C, C], f16, kind="ExternalInput")
    wc_d = nc.dram_tensor("wc", [C, C], f16, kind="ExternalInput")
    lamneg_d = nc.dram_tensor("lamneg", [P, NH], f32, kind="ExternalInput")
    out_d = nc.dram_tensor("out", [T, C], f16, kind="ExternalOutput")

    ln_bias = float(math.log(1.0 - LAMBDA_INIT))
    # fold 2^-6 into the exp so the unnormalized y fits comfortably in f16;
    # the LN scale-invariance (eps scaled by den1^2) makes this exact
    exp_bias = float(-6.0 * math.log(2.0))

    with tile.TileContext(nc) as tc:
        with (
            tc.tile_pool(name="const", bufs=1) as const,
            tc.tile_pool(name="ydata", bufs=8) as y_pool,
            tc.tile_pool(name="vdata", bufs=8) as v_p,
            tc.tile_pool(name="wcp", bufs=8) as wc_p,
        ):
            wc_sb = [wc_p.tile([P, C], f16, tag="wc", name="wcsb") for _ in range(8)]
            dum = const.tile([P, 512], f16, tag="dum")
            nc.gpsimd.memset(dum, 0.0)
            ident = const.tile([P, P], f16, tag="ident")
            make_identity(nc, ident)
            lamneg = const.tile([P, NH], f32, tag="lamneg")
            den_store = const.tile([P, NH, 8], f32, tag="den")
            lnb = const.tile([P, 1], f32, tag="lnb")
            nc.vector.memset(lnb, ln_bias)
            expb = const.tile([P, 1], f32, tag="expb")
            nc.vector.memset(expb, exp_bias)

            y_tiles = [y_pool.tile([P, NH * P], f16, tag="y", name="yt") for _ in range(8)]
            var_tiles = [y_pool.tile([P, NH], f32, tag="var", name="var") for _ in range(8)]
            v_aug = [v_p.tile([P, NH, 132], f16, tag="v", name="vaug") for _ in range(8)]

            p_ctx = tc.tile_pool(name="pprob", bufs=4)
            p_pool = p_ctx.__enter__()
            small_ctx = tc.tile_pool(name="smallc", bufs=16)
            small = small_ctx.__enter__()

            def pv_unit(h, s, i, pcs, ypool, tail=False):
                """PV + stream-combine + LN stats for one q-tile.

                tail=True routes the PSUM drains to the scalar engine (idle
                there) so the DVE never back-pressures the PE via psum WAR.
                """
                n, t = i // 4, i % 4
                pch = pcs[(s, n)]
                yp = ypool.tile([P, 129], f32, tag="psY", name="yp")
                for j in range(i + 1):
                    nc.tensor.matmul(
                        yp,
                        lhsT=pch[:, j, 128 * t : 128 * (t + 1)],
                        rhs=v_aug[j][:, h, 0:129],
                        start=(j == 0),
                        stop=(j == i),
                    )
                ysl = y_tiles[i][:, 128 * h : 128 * (h + 1)]
                if s == 0:
                    if tail:
                        nc.scalar.activation(out=ysl, in_=yp[:, 0:128], func=Act.Copy)
                    else:
                        nc.vector.tensor_copy(out=ysl, in_=yp[:, 0:128])
                    nc.vector.tensor_copy(
                        out=den_store[:, h, i : i + 1], in_=yp[:, 128:129]
                    )
                else:
                    r2 = small.tile([P, 1], f32, tag="r2", name="r2")
                    nc.vector.reciprocal(out=r2, in_=yp[:, 128:129])
                    gneg = small.tile([P, 1], f32, tag="gneg", name="gneg")
                    nc.vector.tensor_mul(
                        out=gneg, in0=den_store[:, h, i : i + 1], in1=r2
                    )
                    nc.vector.tensor_mul(
                        out=gneg, in0=gneg, in1=lamneg[:, h : h + 1]
                    )
                    tmp = small.tile([P, P], f16, tag="tmp", name="tmp")
                    if tail:
                        nc.scalar.activation(
                            out=tmp, in_=yp[:, 0:128], func=Act.Copy, scale=gneg
                        )
                    else:
                        nc.vector.tensor_scalar(
                            out=tmp, in0=yp[:, 0:128], scalar1=gneg, scalar2=None,
                            op0=Alu.mult,
                        )
                    nc.vector.tensor_add(out=ysl, in0=ysl, in1=tmp)
                    bs = small.tile(
                        [P, nc.vector.BN_STATS_DIM], f32, tag="bs", name="bs"
                    )
                    nc.vector.bn_stats(out=bs, in_=ysl)
                    mv = small.tile(
                        [P, nc.vector.BN_AGGR_DIM], f32, tag="mv", name="mv"
                    )
                    nc.vector.bn_aggr(out=mv, in_=bs)
                    nc.vector.tensor_copy(
                        out=var_tiles[i][:, h : h + 1], in_=mv[:, 1:2]
                    )


            def emit_ln(i):
                # veps = var + eps*den1^2 -> invstd via Ln/Exp (both live
                # in the natural_log_exp ACT table set: one switch total)
                d1 = den_store[:, :, i : i + 1].rearrange("p h one -> p (h one)")
                veps = small.tile([P, NH], f32, tag="veps", name="veps")
                nc.vector.tensor_mul(out=veps, in0=d1, in1=d1)
                nc.vector.tensor_scalar(
                    out=veps, in0=veps, scalar1=LN_EPS, scalar2=None,
                    op0=Alu.mult,
                )
                nc.vector.tensor_add(out=veps, in0=veps, in1=var_tiles[i])
                inv = small.tile([P, NH], f32, tag="inv", name="inv")
                nc.scalar.activation(out=inv, in_=veps, func=Act.Ln)
                nc.scalar.activation(
                    out=inv, in_=inv, func=Act.Exp, scale=-0.5, bias=lnb
                )
                # V was host-centered, so mu == 0 and the LN apply is a
                # pure per-row scale; split it across DVE and ACT
                for h in range(NH):
                    ysl = y_tiles[i][:, 128 * h : 128 * (h + 1)]
                    if h % 2 == 0:
                        nc.vector.tensor_scalar(
                            out=ysl, in0=ysl,
                            scalar1=inv[:, h : h + 1], scalar2=None,
                            op0=Alu.mult,
                        )
                    else:
                        nc.scalar.activation(
                            out=ysl, in_=ysl, func=Act.Copy,
                            scale=inv[:, h : h + 1],
                        )

            # ---------- merged projections + attention ----------
            with (
                tc.tile_pool(name="xT", bufs=8) as xT_p,
                tc.tile_pool(name="wv", bufs=8) as wv_p,
                tc.tile_pool(name="wqk", bufs=2) as wqk_p,
                tc.tile_pool(name="qk", bufs=2) as qk_p,
                tc.tile_pool(name="psB2", bufs=2, space="PSUM") as psB2,
                tc.tile_pool(name="psS", bufs=4, space="PSUM") as psS,
                tc.tile_pool(name="psY", bufs=2, space="PSUM") as psY,
            ):
                xT = [xT_p.tile([P, T], f16, tag="xT", name="xT") for _ in range(8)]
                wv_sb = [wv_p.tile([P, C], f16, tag="w", name="wsb") for _ in range(8)]

                def emit_wdma(h, eng=None):
                    """One strided DMA per weight matrix for head h:
                    out[p, c, d] = w[128c + p, 128h + d]."""
                    tiles = []
                    for w_d, tag, nm in ((wq_d, "wq", "wqh"), (wk_d, "wk", "wkh")):
                        wt = wqk_p.tile([P, 8, P], f16, tag=tag, name=nm)
                        src_ap = w_d.rearrange("(c p) d -> p c d", p=P)[
                            :, :, 128 * h : 128 * (h + 1)
                        ]
                        (eng or nc.sync).dma_start(out=wt, in_=src_ap)
                        tiles.append(wt)
                    return tiles

                def emit_proj(wt, dest):
                    """(head_dim 128, T) projection for one head."""
                    for n in range(2):
                        ps = psB2.tile([P, 512], f32, tag="psB2", name="pps")
                        for c in range(8):
                            nc.tensor.matmul(
                                ps,
                                lhsT=wt[:, c, :],
                                rhs=xT[c][:, 512 * n : 512 * (n + 1)],
                                start=(c == 0),
                                stop=(c == 7),
                            )
                        nc.vector.tensor_copy(
                            out=dest[:, 512 * n : 512 * (n + 1)], in_=ps
                        )

                def score_unit(h, qT, kT, pcs, n, j):
                    """One k-tile of scores: both streams' matmuls in
                    concurrent PE row-groups, then exp + diag mask."""
                    qlo = 128 * max(0, j - 4 * n)
                    sp2 = [
                        psS.tile([P, 512], f32, tag="psS", name="sp")
                        for _ in range(2)
                    ]
                    for s in range(2):
                        nc.tensor.matmul(
                            sp2[s][:, qlo:512],
                            lhsT=kT[64 * s : 64 * (s + 1), 128 * j : 128 * (j + 1)],
                            rhs=qT[
                                64 * s : 64 * (s + 1),
                                512 * n + qlo : 512 * (n + 1),
                            ],
                            start=True,
                            stop=True,
                        )
                    t = j - 4 * n
                    for s in range(2):
                        pch = pcs[(s, n)]
                        nc.scalar.activation(
                            out=pch[:, j, qlo:512],
                            in_=sp2[s][:, qlo:512],
                            func=Act.Exp,
                            scale=0.125,
                            bias=expb,
                        )
                        if 0 <= t <= 3:
                            nc.gpsimd.affine_select(
                                out=pch[:, j, 128 * t : 128 * (t + 1)],
                                in_=pch[:, j, 128 * t : 128 * (t + 1)],
                                compare_op=Alu.is_ge,
                                fill=0.0,
                                base=0,
                                pattern=[[1, 128]],
                                channel_multiplier=-1,
                            )

                def vproj_unit(t):
                    """V-projection tile, both 512-col halves."""
                    for n in range(2):
                        ps = psB2.tile([P, 512], f32, tag="psB2", name="pps")
                        for c in range(8):
                            nc.tensor.matmul(
                                ps,
                                lhsT=xT[c][:, 128 * t : 128 * (t + 1)],
                                rhs=wv_sb[c][:, 512 * n : 512 * (n + 1)],
                                start=(c == 0),
                                stop=(c == 7),
                            )
                        nc.any.tensor_copy(
                            out=v_aug[t][:, 4 * n : 4 * (n + 1), 0:128],
                            in_=ps.rearrange("p (g d) -> p g d", g=4),
                        )
                    nc.vector.memset(v_aug[t][:, :, 128:129], 1.0)

                # software pipeline: PV matmuls of head h-1 (and the
                # V-projection, for h=0) are interleaved between head h's
                # score units so PE never stalls on exp
                # PE warm-up: dummy matmuls with no input deps keep the
                # HAM clock at 8/8 while the initial DMAs land
                def emit_dummies(k):
                    for _ in range(k):
                        scr = psS.tile([P, 512], f32, tag="psS", name="sp")
                        for _ in range(8):
                            nc.tensor.matmul(
                                scr, lhsT=dum[:, 0:128], rhs=dum,
                                start=True, stop=True,
                            )

                emit_dummies(2)

                # critical path first: head-0 weights + xT split over the
                # sync/scalar queues; wv owns the gpsimd queue (needed later)
                wt0 = wqk_p.tile([P, 8, P], f16, tag="wq", name="wqh")
                nc.sync.dma_start(
                    out=wt0,
                    in_=wq_d.rearrange("(c p) d -> p c d", p=P)[:, :, 0:128],
                )
                wt1 = wqk_p.tile([P, 8, P], f16, tag="wk", name="wkh")
                nc.scalar.dma_start(
                    out=wt1,
                    in_=wk_d.rearrange("(c p) d -> p c d", p=P)[:, :, 0:128],
                )
                wts = [wt0, wt1]
                # DMA priority: xT first (gates head-0 projections), then wv
                # (needed ~15us in, during head-0's score phase), then wc
                # (needed only at the tail) — all on the sync/scalar queue
                # pair so the later loads cannot steal HBM bandwidth from xT
                for c in range(8):
                    eng = nc.sync if c % 2 == 0 else nc.scalar
                    eng.dma_start(out=xT[c], in_=xt_d[128 * c : 128 * (c + 1), :])
                for c in range(8):
                    eng = nc.sync if c % 2 == 0 else nc.scalar
                    eng.dma_start(out=wv_sb[c], in_=wv_d[128 * c : 128 * (c + 1), :])
                for dd in range(8):
                    eng = nc.sync if dd % 2 == 0 else nc.scalar
                    eng.dma_start(out=wc_sb[dd], in_=wc_d[128 * dd : 128 * (dd + 1), :])
                nc.gpsimd.dma_start(out=lamneg, in_=lamneg_d[:, :])
                pcs_prev = None
                for h in range(NH):
                    qT = qk_p.tile([P, T], f16, tag="q", name="qT")
                    kT = qk_p.tile([P, T], f16, tag="k", name="kT")
                    if h == 0:
                        # head 0 is paced by the xT DMA: interleave Q and K
                        # per c-chunk so each chunk is consumed as it lands
                        # (K's accumulators borrow psS - scores idle so far)
                        psq = [
                            psB2.tile([P, 512], f32, tag="psB2", name="pq")
                            for _ in range(2)
                        ]
                        psk = [
                            psS.tile([P, 512], f32, tag="psS", name="pk")
                            for _ in range(2)
                        ]
                        for c in range(8):
                            for n in range(2):
                                nc.tensor.matmul(
                                    psq[n],
                                    lhsT=wts[0][:, c, :],
                                    rhs=xT[c][:, 512 * n : 512 * (n + 1)],
                                    start=(c == 0),
                                    stop=(c == 7),
                                )
                            for n in range(2):
                                nc.tensor.matmul(
                                    psk[n],
                                    lhsT=wts[1][:, c, :],
                                    rhs=xT[c][:, 512 * n : 512 * (n + 1)],
                                    start=(c == 0),
                                    stop=(c == 7),
                                )
                        for n in range(2):
                            nc.vector.tensor_copy(
                                out=qT[:, 512 * n : 512 * (n + 1)], in_=psq[n]
                            )
                            nc.vector.tensor_copy(
                                out=kT[:, 512 * n : 512 * (n + 1)], in_=psk[n]
                            )
                    else:
                        emit_proj(wts[0], qT)
                        emit_proj(wts[1], kT)
                    if h + 1 < NH:
                        next_wts = emit_wdma(h + 1)
                    pcs = {
                        (s, n): p_pool.tile(
                            [P, 4 * n + 4, 512], f16,
                            tag=f"p{n}", name="pch", bufs=4,
                        )
                        for s in range(2)
                        for n in range(2)
                    }
                    if h == 0:
                        backlog = [("v", t) for t in range(8)]
                    else:
                        backlog = [("pv", s, i) for i in range(8) for s in range(2)]
                    if h == NH - 1:
                        # head 7 has no successor iteration: pull its first
                        # four q-tiles' PV units into this head's backlog,
                        # spread through the drain order as their exp chunks
                        # become available
                        for pos, ti in ((8, 3), (12, 2), (16, 1), (20, 0)):
                            backlog[pos:pos] = [("pv7", 0, ti), ("pv7", 1, ti)]
                    sunits = [(n, j) for n in range(2) for j in range(4 * n + 4)]
                    done = 0
                    for idx, (n, j) in enumerate(sunits):
                        score_unit(h, qT, kT, pcs, n, j)
                        while done < len(backlog) and (idx + 1) * len(
                            backlog
                        ) >= (done + 1) * len(sunits):
                            u = backlog[done]
                            done += 1
                            # late head-7 backlog units drain on ACT: exp is
                            # finished there, and the DVE otherwise paces the
                            # PE via psum WAR with no score units left to fill
                            late = h == NH - 1 and done > 16
                            if u[0] == "v":
                                vproj_unit(u[1])
                            elif u[0] == "pv7":
                                pv_unit(h, u[1], u[2], pcs, psY, tail=late)
                            else:
                                pv_unit(h - 1, u[1], u[2], pcs_prev, psY, tail=late)
                    while done < len(backlog):
                        u = backlog[done]
                        done += 1
                        late = h == NH - 1 and done > 16
                        if u[0] == "v":
                            vproj_unit(u[1])
                        elif u[0] == "pv7":
                            pv_unit(h, u[1], u[2], pcs, psY, tail=late)
                        else:
                            pv_unit(h - 1, u[1], u[2], pcs_prev, psY, tail=late)
                    if h == NH - 1:
                        # tiles 3 and 2 finished PV above: run their LN now
                        # so the tail's first transposes never wait on it
                        emit_ln(3)
                        emit_ln(2)
                        # filler matmuls bridge the pool-transition stall so
                        # the HAM activity window never sees the PE idle
                        emit_dummies(1)
                    pcs_prev = pcs
                    if h + 1 < NH:
                        wts = next_wts

            # ---------- tail: PV(7) + LN + transpose pipeline, then c_proj ----------
            with (
                tc.tile_pool(name="smalld", bufs=10) as sd,
                tc.tile_pool(name="ylnT", bufs=8) as ylnT_p,
                tc.tile_pool(name="outp", bufs=3) as out_p,
                tc.tile_pool(name="psY2", bufs=3, space="PSUM") as psY2,
                tc.tile_pool(name="psE", bufs=3, space="PSUM") as psE,
                tc.tile_pool(name="psF", bufs=2, space="PSUM") as psF,
            ):
                ylnT = [ylnT_p.tile([P, T], f16, tag="ylnT", name="ylnT") for _ in range(8)]

                def emit_trans(i):
                    for dh in range(2):
                        pt = psE.tile([P, 512], f16, tag="psE")
                        for w in range(4):
                            dd = 4 * dh + w
                            nc.tensor.transpose(
                                out=pt[:, 128 * w : 128 * (w + 1)],
                                in_=y_tiles[i][:, 128 * dd : 128 * (dd + 1)],
                                identity=ident,
                            )
                        for w in range(4):
                            dd = 4 * dh + w
                            nc.any.tensor_copy(
                                out=ylnT[dd][:, 128 * i : 128 * (i + 1)],
                                in_=pt[:, 128 * w : 128 * (w + 1)],
                            )

                def emit_cproj(i, last=False):
                    # c_proj row-block m = i only needs this q-tile's
                    # transposed columns; the final tile uses 256-col groups
                    # to shorten the end-of-kernel dependency chain
                    osb = out_p.tile([P, C], f16, tag="osb")
                    ngrp, w = (4, 256) if last else (2, 512)
                    for n in range(ngrp):
                        ps = psF.tile([P, w], f32, tag="psF", name="psn")
                        for d in range(8):
                            nc.tensor.matmul(
                                ps,
                                lhsT=ylnT[d][:, 128 * i : 128 * (i + 1)],
                                rhs=wc_sb[d][:, w * n : w * (n + 1)],
                                start=(d == 0),
                                stop=(d == 7),
                            )
                        if last and n % 2 == 1:
                            nc.scalar.activation(
                                out=osb[:, w * n : w * (n + 1)],
                                in_=ps, func=Act.Copy,
                            )
                        else:
                            nc.any.tensor_copy(
                                out=osb[:, w * n : w * (n + 1)], in_=ps
                            )
                        nc.sync.dma_start(
                            out=out_d[128 * i : 128 * (i + 1), w * n : w * (n + 1)],
                            in_=osb[:, w * n : w * (n + 1)],
                        )


                # q-tiles 0-3 finished PV inside the head-7 loop: emit their
                # LN chains first (pure DVE/ACT), then interleave the
                # remaining PV units with their trans+cproj so the PE never
                # waits on the LN latency
                # LN chains are emitted just-in-time (two per iteration) so
                # the DVE queue never delays the tail PV psum drains, which
                # would stall the PE via psum WAR
                ready = [3, 2, 1, 0]
                jit = [1, 0, None, None]
                for k, i in enumerate([7, 6, 5, 4]):
                    for s in range(2):
                        pv_unit(NH - 1, s, i, pcs_prev, psY2)
                    # trans+cproj first: its DVE copies must not queue behind
                    # the LN chains in the vector engine's FIFO
                    emit_trans(ready[k])
                    emit_cproj(ready[k])
                    emit_ln(i)
                    if jit[k] is not None:
                        emit_ln(jit[k])
                # software-pipeline the final four tiles: tile i+1's
                # transposes (and their copies) run under tile i's c_proj
                emit_trans(7)
                emit_trans(6)
                emit_cproj(7)
                emit_trans(5)
                emit_cproj(6)
                emit_trans(4)
                emit_cproj(5)
                emit_cproj(4, last=True)

            small_ctx.__exit__(None, None, None)
            p_ctx.__exit__(None, None, None)

    bass._bass_rust.generate_event_semaphores(nc)
    return nc


_NC = None


def _get_program():
    global _NC
    if _NC is None:
        _NC = build_program()
    return _NC


def make_in_maps(inputs):
    """Host-side sharding: per-core input dicts."""
    x = np.ascontiguousarray(np.asarray(inputs["x"], dtype=np.float32))
    Wq1 = np.asarray(inputs["Wq1"], dtype=np.float32)
    Wq2 = np.asarray(inputs["Wq2"], dtype=np.float32)
    Wk1 = np.asarray(inputs["Wk1"], dtype=np.float32)
    Wk2 = np.asarray(inputs["Wk2"], dtype=np.float32)
    Wv = np.asarray(inputs["Wv"], dtype=np.float32)
    Wc = np.asarray(inputs["Wc"], dtype=np.float32)
    lq1 = np.asarray(inputs["lq1"], dtype=np.float32)
    lk1 = np.asarray(inputs["lk1"], dtype=np.float32)
    lq2 = np.asarray(inputs["lq2"], dtype=np.float32)
    lk2 = np.asarray(inputs["lk2"], dtype=np.float32)

    lam1 = np.exp(np.sum(lq1 * lk1, axis=-1))
    lam2 = np.exp(np.sum(lq2 * lk2, axis=-1))
    lam_full = (lam1 - lam2 + LAMBDA_INIT).astype(np.float32)  # (16,)

    in_maps = []
    for core in range(N_CORES):
        b, hg = core // 2, core % 2
        heads = np.arange(NH) + NH * hg  # global head idx
        wq = np.empty((C, C), np.float32)
        wk = np.empty((C, C), np.float32)
        wv = np.empty((C, C), np.float32)
        for h in range(NH):
            H = NH * hg + h
            wq[:, 128 * h : 128 * h + 64] = Wq1[:, HS * H : HS * (H + 1)]
            wq[:, 128 * h + 64 : 128 * (h + 1)] = Wq2[:, HS * H : HS * (H + 1)]
            wk[:, 128 * h : 128 * h + 64] = Wk1[:, HS * H : HS * (H + 1)]
            wk[:, 128 * h + 64 : 128 * (h + 1)] = Wk2[:, HS * H : HS * (H + 1)]
            # center each head's V block: y = att @ V is then exactly
            # row-mean-free, so the LN mean subtraction vanishes (mu == 0)
            blk = Wv[:, 128 * H : 128 * (H + 1)]
            wv[:, 128 * h : 128 * (h + 1)] = blk - blk.mean(axis=1, keepdims=True)
        wc = np.ascontiguousarray(Wc[1024 * hg : 1024 * (hg + 1), :]).astype(np.float16)
        lamneg = np.broadcast_to(
            -lam_full[heads][None, :], (P, NH)
        ).astype(np.float32)
        in_maps.append(
            {
                "xt": np.ascontiguousarray(x[b].T.astype(np.float16)),
                "wq": wq.astype(np.float16),
                "wk": wk.astype(np.float16),
                "wv": wv.astype(np.float16),
                "wc": wc,
                "lamneg": np.ascontiguousarray(lamneg),
            }
        )
    return in_maps


def run(inputs, trace=False, **kw):
    from concourse.bass_utils import run_bass_kernel_spmd

    nc = _get_program()
    in_maps = make_in_maps(inputs)
    res = run_bass_kernel_spmd(
        nc, in_maps, core_ids=list(range(N_CORES)), trace=trace, **kw
    )
    B = 4
    out = np.empty((B, T, C), np.float32)
    for b in range(B):
        out[b] = (
            res.results[2 * b]["out"].astype(np.float32)
            + res.results[2 * b + 1]["out"].astype(np.float32)
        )
    return out, res


def kernel(**inputs) -> np.ndarray:
    out, _ = run(inputs, trace=False)
    return out



# revision 55
# speedup vs baseline: 1.0144x; 1.0140x over previous
"""MultiHeadDiffAttention Trainium2 kernel (8 NeuronCores).

Sharding: batch (4) x head-group (2 groups of 8 heads) = 8 cores.
Each core computes a partial (T, C) c_proj output for its batch element
restricted to its 8 heads; the host sums the two head-group f16 partials
per batch element in f32.

Per-core pipeline (all matmuls on PE in fp16, PSUM accumulation fp32):
  1. Host pre-transposes x[b] -> xT (C on partitions) and interleaves the
     Q/K weights so each 128-col chunk is one head's [q1|q2] dims. The
     per-head V blocks of Wv are column-mean-centered on the host, which
     makes y = att @ V exactly row-mean-free: the LayerNorm mean
     subtraction vanishes (mu == 0) and the LN apply is a pure scale.
  2. DMA priority xT -> wv -> wc on the sync/scalar queue pair (the
     startup is HBM-bandwidth-bound); dummy matmuls keep the PE HAM
     clock at 8/8 until xT lands.
  3. Per head/stream: scores S^T(k,q) = K^T-tiles x Q^T (contract d=64),
     exp via ScalarE with fused scale 1/8 and bias -6*ln2 (the 2^-6
     keeps the unnormalized y in f16 range; LN scale-invariance makes it
     exact) -> fp16 P, causal mask via gpsimd affine_select, then PV:
     P-tile^T x [V|1] accumulated over k-tiles gives Y and the softmax
     denominator in one matmul. PV of head h-1 (and the V projection,
     for h=0) is interleaved between head h's score units; head 7's
     first four q-tiles' PV rides inside head 7's own iteration.
  4. Streams combined as z = Y1 - (lam*den1/den2) * Y2 (per-q scalars),
     which equals den1 * (a1 - lam*a2) @ V; LayerNorm is scale-invariant
     per row, so normalizing z with eps scaled by den1^2 reproduces the
     reference exactly.
  5. LN invstd via bn_stats/bn_aggr + exp(-0.5*ln(var+eps*den1^2) +
     ln(1-li)); the apply (a row scale) is split across DVE and ACT.
     LN chains are emitted just-in-time so the DVE FIFO never delays
     the tail PV psum drains (PE stalls via psum WAR otherwise, and any
     >~25% idle HAM window re-throttles the PE clock to 1.2 GHz).
  6. f16 PE-transpose of y_ln (f16 PSUM), c_proj vs host-sliced f16 Wc
     rows -> f16 partial out, with tile i+1's transposes pipelined under
     tile i's c_proj and the final tile in 256-col groups to shorten the
     end-of-kernel chain.
"""

import contextlib
import ctypes
import math
import sys
import types

import numpy as np

sys.path.insert(0, "/opt/trn_rl_repo")


def _install_ntff_hook():
    """Provide antenv.axon_hooks if the image lacks it (for trace=True)."""
    try:
        from antenv.axon_hooks import get_axon_ntff_profile_hook  # noqa: F401

        return
    except ImportError:
        pass

    so_path = "/opt/axon/libaxon_pjrt.so"

    def _make_hook():
        try:
            lib = ctypes.CDLL(so_path)
        except OSError:
            return None
        if not hasattr(lib, "axon_start_nrt_profile"):
            return None
        lib.axon_start_nrt_profile.argtypes = [
            ctypes.POINTER(ctypes.c_int64),
            ctypes.c_size_t,
        ]
        lib.axon_start_nrt_profile.restype = ctypes.c_int64
        lib.axon_stop_nrt_profile.argtypes = [ctypes.c_char_p]
        lib.axon_stop_nrt_profile.restype = ctypes.c_int64

        @contextlib.contextmanager
        def _hook(output_dir, device_ids):
            import jax

            jax.devices()
            if device_ids:
                ids = (ctypes.c_int64 * len(device_ids))(*device_ids)
                rc = lib.axon_start_nrt_profile(ids, len(device_ids))
            else:
                rc = lib.axon_start_nrt_profile(None, 0)
            if rc != 0:
                raise RuntimeError(f"axon_start_nrt_profile rc={rc}")
            try:
                yield
            finally:
                n = lib.axon_stop_nrt_profile(str(output_dir).encode())
                if n < 0:
                    raise RuntimeError(f"axon_stop_nrt_profile rc={n}")

        return _hook

    mod = types.ModuleType("antenv.axon_hooks")
    _the_hook = _make_hook()
    mod.get_axon_ntff_profile_hook = lambda: _the_hook
    sys.modules["antenv.axon_hooks"] = mod


_install_ntff_hook()

import concourse.bass as bass  # noqa: E402
import concourse.bass_utils as bass_utils_mod  # noqa: E402
import concourse.mybir as mybir  # noqa: E402
import concourse.tile as tile  # noqa: E402
from concourse.masks import make_identity  # noqa: E402


def _enable_ldw_opt():
    """walrus ships with --enable-ldw-opt=false hardcoded; the LDWEIGHTS
    optimization pass overlaps weight loads with in-flight matmuls, which
    matters a lot for this kernel (a fresh stationary operand per matmul
    in the PV stage). Rewrite the flag on the walrus command line."""
    orig = bass_utils_mod.run_command

    def patched(argv, **kwargs):
        argv = [
            "--enable-ldw-opt=true" if a == "--enable-ldw-opt=false" else a
            for a in argv
        ]
        return orig(argv, **kwargs)

    bass_utils_mod.run_command = patched


import os  # noqa: E402

if os.environ.get("BASS_LDW_OPT", "0") == "1":
    _enable_ldw_opt()

P = 128
T = 1024
C = 1024
NH = 8  # heads per core
HS = 64
LAMBDA_INIT = 0.8 - 0.6 * math.exp(-0.3 * (2 - 1))
LN_EPS = 1e-5
N_CORES = 8

f32 = mybir.dt.float32
f32r = mybir.dt.float32r
f16 = mybir.dt.float16
Alu = mybir.AluOpType
Act = mybir.ActivationFunctionType


def r(ap):
    return ap.bitcast(f32r)


def build_program():
    nc = bass.Bass()
    xt_d = nc.dram_tensor("xt", [C, T], f16, kind="ExternalInput")
    wq_d = nc.dram_tensor("wq", [C, C], f16, kind="ExternalInput")
    wk_d = nc.dram_tensor("wk", [C, C], f16, kind="ExternalInput")
    wv_d = nc.dram_tensor("wv", [C, C], f16, kind="ExternalInput")
    wc_d = nc.dram_tensor("wc", [C, C], f16, kind="ExternalInput")
    lamneg_d = nc.dram_tensor("lamneg", [P, NH], f32, kind="ExternalInput")
    out_d = nc.dram_tensor("out", [T, C], f16, kind="ExternalOutput")

    ln_bias = float(math.log(1.0 - LAMBDA_INIT))
    # fold 2^-6 into the exp so the unnormalized y fits comfortably in f16;
    # the LN scale-invariance (eps scaled by den1^2) makes this exact
    exp_bias = float(-6.0 * math.log(2.0))

    with tile.TileContext(nc) as tc:
        with (
            tc.tile_pool(name="const", bufs=1) as const,
            tc.tile_pool(name="ydata", bufs=8) as y_pool,
            tc.tile_pool(name="vdata", bufs=8) as v_p,
            tc.tile_pool(name="wcp", bufs=8) as wc_p,
        ):
            wc_sb = [wc_p.tile([P, C], f16, tag="wc", name="wcsb") for _ in range(8)]
            dum = const.tile([P, 512], f16, tag="dum")
            nc.gpsimd.memset(dum, 0.0)
            ident = const.tile([P, P], f16, tag="ident")
            make_identity(nc, ident)
            lamneg = const.tile([P, NH], f32, tag="lamneg")
            den_store = const.tile([P, NH, 8], f32, tag="den")
            lnb = const.tile([P, 1], f32, tag="lnb")
            nc.vector.memset(lnb, ln_bias)
            expb = const.tile([P, 1], f32, tag="expb")
            nc.vector.memset(expb, exp_bias)

            y_tiles = [y_pool.tile([P, NH * P], f16, tag="y", name="yt") for _ in range(8)]
            var_tiles = [y_pool.tile([P, NH], f32, tag="var", name="var") for _ in range(8)]
            v_aug = [v_p.tile([P, NH, 132], f16, tag="v", name="vaug") for _ in range(8)]

            p_ctx = tc.tile_pool(name="pprob", bufs=4)
            p_pool = p_ctx.__enter__()
            small_ctx = tc.tile_pool(name="smallc", bufs=16)
            small = small_ctx.__enter__()

            def pv_unit(h, s, i, pcs, ypool, tail=False):
                """PV + stream-combine + LN stats for one q-tile.

                tail=True routes the PSUM drains to the scalar engine (idle
                there) so the DVE never back-pressures the PE via psum WAR.
                """
                n, t = i // 4, i % 4
                pch = pcs[(s, n)]
                yp = ypool.tile([P, 129], f32, tag="psY", name="yp")
                for j in range(i + 1):
                    nc.tensor.matmul(
                        yp,
                        lhsT=pch[:, j, 128 * t : 128 * (t + 1)],
                        rhs=v_aug[j][:, h, 0:129],
                        start=(j == 0),
                        stop=(j == i),
                    )
                ysl = y_tiles[i][:, 128 * h : 128 * (h + 1)]
                if s == 0:
                    if tail:
                        nc.scalar.activation(out=ysl, in_=yp[:, 0:128], func=Act.Copy)
                    else:
                        nc.vector.tensor_copy(out=ysl, in_=yp[:, 0:128])
                    nc.vector.tensor_copy(
                        out=den_store[:, h, i : i + 1], in_=yp[:, 128:129]
                    )
                else:
                    r2 = small.tile([P, 1], f32, tag="r2", name="r2")
                    nc.vector.reciprocal(out=r2, in_=yp[:, 128:129])
                    gneg = small.tile([P, 1], f32, tag="gneg", name="gneg")
                    nc.vector.tensor_mul(
                        out=gneg, in0=den_store[:, h, i : i + 1], in1=r2
                    )
                    nc.vector.tensor_mul(
                        out=gneg, in0=gneg, in1=lamneg[:, h : h + 1]
                    )
                    tmp = small.tile([P, P], f16, tag="tmp", name="tmp")
                    if tail:
                        nc.scalar.activation(
                            out=tmp, in_=yp[:, 0:128], func=Act.Copy, scale=gneg
                        )
                    else:
                        nc.vector.tensor_scalar(
                            out=tmp, in0=yp[:, 0:128], scalar1=gneg, scalar2=None,
                            op0=Alu.mult,
                        )
                    nc.vector.tensor_add(out=ysl, in0=ysl, in1=tmp)
                    bs = small.tile(
                        [P, nc.vector.BN_STATS_DIM], f32, tag="bs", name="bs"
                    )
                    nc.vector.bn_stats(out=bs, in_=ysl)
                    mv = small.tile(
                        [P, nc.vector.BN_AGGR_DIM], f32, tag="mv", name="mv"
                    )
                    nc.vector.bn_aggr(out=mv, in_=bs)
                    nc.vector.tensor_copy(
                        out=var_tiles[i][:, h : h + 1], in_=mv[:, 1:2]
                    )


            def emit_ln(i):
                # veps = var + eps*den1^2 -> invstd via Ln/Exp (both live
                # in the natural_log_exp ACT table set: one switch total)
                d1 = den_store[:, :, i : i + 1].rearrange("p h one -> p (h one)")
                veps = small.tile([P, NH], f32, tag="veps", name="veps")
                nc.vector.tensor_mul(out=veps, in0=d1, in1=d1)
                nc.vector.tensor_scalar(
                    out=veps, in0=veps, scalar1=LN_EPS, scalar2=None,
                    op0=Alu.mult,
                )
                nc.vector.tensor_add(out=veps, in0=veps, in1=var_tiles[i])
                inv = small.tile([P, NH], f32, tag="inv", name="inv")
                nc.scalar.activation(out=inv, in_=veps, func=Act.Ln)
                nc.scalar.activation(
                    out=inv, in_=inv, func=Act.Exp, scale=-0.5, bias=lnb
                )
                # V was host-centered, so mu == 0 and the LN apply is a
                # pure per-row scale; split it across DVE and ACT
                for h in range(NH):
                    ysl = y_tiles[i][:, 128 * h : 128 * (h + 1)]
                    if h % 2 == 0:
                        nc.vector.tensor_scalar(
                            out=ysl, in0=ysl,
                            scalar1=inv[:, h : h + 1], scalar2=None,
                            op0=Alu.mult,
                        )
                    else:
                        nc.scalar.activation(
                            out=ysl, in_=ysl, func=Act.Copy,
                            scale=inv[:, h : h + 1],
                        )

            # ---------- merged projections + attention ----------
            with (
                tc.tile_pool(name="xT", bufs=8) as xT_p,
                tc.tile_pool(name="wv", bufs=8) as wv_p,
                tc.tile_pool(name="wqk", bufs=2) as wqk_p,
                tc.tile_pool(name="qk", bufs=2) as qk_p,
                tc.tile_pool(name="psB2", bufs=2, space="PSUM") as psB2,
                tc.tile_pool(name="psS", bufs=4, space="PSUM") as psS,
                tc.tile_pool(name="psY", bufs=2, space="PSUM") as psY,
            ):
                xT = [xT_p.tile([P, T], f16, tag="xT", name="xT") for _ in range(8)]
                wv_sb = [wv_p.tile([P, C], f16, tag="w", name="wsb") for _ in range(8)]

                def emit_wdma(h, eng=None):
                    """One strided DMA per weight matrix for head h:
                    out[p, c, d] = w[128c + p, 128h + d]."""
                    tiles = []
                    for w_d, tag, nm in ((wq_d, "wq", "wqh"), (wk_d, "wk", "wkh")):
                        wt = wqk_p.tile([P, 8, P], f16, tag=tag, name=nm)
                        src_ap = w_d.rearrange("(c p) d -> p c d", p=P)[
                            :, :, 128 * h : 128 * (h + 1)
                        ]
                        (eng or nc.sync).dma_start(out=wt, in_=src_ap)
                        tiles.append(wt)
                    return tiles

                def emit_proj(wt, dest):
                    """(head_dim 128, T) projection for one head."""
                    for n in range(2):
                        ps = psB2.tile([P, 512], f32, tag="psB2", name="pps")
                        for c in range(8):
                            nc.tensor.matmul(
                                ps,
                                lhsT=wt[:, c, :],
                                rhs=xT[c][:, 512 * n : 512 * (n + 1)],
                                start=(c == 0),
                                stop=(c == 7),
                            )
                        nc.vector.tensor_copy(
                            out=dest[:, 512 * n : 512 * (n + 1)], in_=ps
                        )

                def score_unit(h, qT, kT, pcs, n, j):
                    """One k-tile of scores: both streams' matmuls in
                    concurrent PE row-groups, then exp + diag mask."""
                    qlo = 128 * max(0, j - 4 * n)
                    sp2 = [
                        psS.tile([P, 512], f32, tag="psS", name="sp")
                        for _ in range(2)
                    ]
                    for s in range(2):
                        nc.tensor.matmul(
                            sp2[s][:, qlo:512],
                            lhsT=kT[64 * s : 64 * (s + 1), 128 * j : 128 * (j + 1)],
                            rhs=qT[
                                64 * s : 64 * (s + 1),
                                512 * n + qlo : 512 * (n + 1),
                            ],
                            start=True,
                            stop=True,
                        )
                    t = j - 4 * n
                    for s in range(2):
                        pch = pcs[(s, n)]
                        nc.scalar.activation(
                            out=pch[:, j, qlo:512],
                            in_=sp2[s][:, qlo:512],
                            func=Act.Exp,
                            scale=0.125,
                            bias=expb,
                        )
                        if 0 <= t <= 3:
                            nc.gpsimd.affine_select(
                                out=pch[:, j, 128 * t : 128 * (t + 1)],
                                in_=pch[:, j, 128 * t : 128 * (t + 1)],
                                compare_op=Alu.is_ge,
                                fill=0.0,
                                base=0,
                                pattern=[[1, 128]],
                                channel_multiplier=-1,
                            )

                def vproj_unit(t):
                    """V-projection tile, both 512-col halves."""
                    for n in range(2):
                        ps = psB2.tile([P, 512], f32, tag="psB2", name="pps")
                        for c in range(8):
                            nc.tensor.matmul(
                                ps,
                                lhsT=xT[c][:, 128 * t : 128 * (t + 1)],
                                rhs=wv_sb[c][:, 512 * n : 512 * (n + 1)],
                                start=(c == 0),
                                stop=(c == 7),
                            )
                        nc.any.tensor_copy(
                            out=v_aug[t][:, 4 * n : 4 * (n + 1), 0:128],
                            in_=ps.rearrange("p (g d) -> p g d", g=4),
                        )
                    nc.vector.memset(v_aug[t][:, :, 128:129], 1.0)

                # software pipeline: PV matmuls of head h-1 (and the
                # V-projection, for h=0) are interleaved between head h's
                # score units so PE never stalls on exp
                # PE warm-up: dummy matmuls with no input deps keep the
                # HAM clock at 8/8 while the initial DMAs land
                def emit_dummies(k):
                    for _ in range(k):
                        scr = psS.tile([P, 512], f32, tag="psS", name="sp")
                        for _ in range(8):
                            nc.tensor.matmul(
                                scr, lhsT=dum[:, 0:128], rhs=dum,
                                start=True, stop=True,
                            )

                emit_dummies(2)

                # critical path first: head-0 weights + xT split over the
                # sync/scalar queues; wv owns the gpsimd queue (needed later)
                wt0 = wqk_p.tile([P, 8, P], f16, tag="wq", name="wqh")
                nc.sync.dma_start(
                    out=wt0,
                    in_=wq_d.rearrange("(c p) d -> p c d", p=P)[:, :, 0:128],
                )
                wt1 = wqk_p.tile([P, 8, P], f16, tag="wk", name="wkh")
                nc.scalar.dma_start(
                    out=wt1,
                    in_=wk_d.rearrange("(c p) d -> p c d", p=P)[:, :, 0:128],
                )
                wts = [wt0, wt1]
                # DMA priority: xT first (gates head-0 projections), then wv
                # (needed ~15us in, during head-0's score phase), then wc
                # (needed only at the tail) — all on the sync/scalar queue
                # pair so the later loads cannot steal HBM bandwidth from xT
                for c in range(8):
                    eng = nc.sync if c % 2 == 0 else nc.scalar
                    eng.dma_start(out=xT[c], in_=xt_d[128 * c : 128 * (c + 1), :])
                for c in range(8):
                    eng = nc.sync if c % 2 == 0 else nc.scalar
                    eng.dma_start(out=wv_sb[c], in_=wv_d[128 * c : 128 * (c + 1), :])
                for dd in range(8):
                    eng = nc.sync if dd % 2 == 0 else nc.scalar
                    eng.dma_start(out=wc_sb[dd], in_=wc_d[128 * dd : 128 * (dd + 1), :])
                nc.gpsimd.dma_start(out=lamneg, in_=lamneg_d[:, :])
                pcs_prev = None
                for h in range(NH):
                    qT = qk_p.tile([P, T], f16, tag="q", name="qT")
                    kT = qk_p.tile([P, T], f16, tag="k", name="kT")
                    if h == 0:
                        # head 0 is paced by the xT DMA: interleave Q and K
                        # per c-chunk so each chunk is consumed as it lands
                        # (K's accumulators borrow psS - scores idle so far)
                        psq = [
                            psB2.tile([P, 512], f32, tag="psB2", name="pq")
                            for _ in range(2)
                        ]
                        psk = [
                            psS.tile([P, 512], f32, tag="psS", name="pk")
                            for _ in range(2)
                        ]
                        for c in range(8):
                            for n in range(2):
                                nc.tensor.matmul(
                                    psq[n],
                                    lhsT=wts[0][:, c, :],
                                    rhs=xT[c][:, 512 * n : 512 * (n + 1)],
                                    start=(c == 0),
                                    stop=(c == 7),
                                )
                            for n in range(2):
                                nc.tensor.matmul(
                                    psk[n],
                                    lhsT=wts[1][:, c, :],
                                    rhs=xT[c][:, 512 * n : 512 * (n + 1)],
                                    start=(c == 0),
                                    stop=(c == 7),
                                )
                        for n in range(2):
                            nc.vector.tensor_copy(
                                out=qT[:, 512 * n : 512 * (n + 1)], in_=psq[n]
                            )
                            nc.vector.tensor_copy(
                                out=kT[:, 512 * n : 512 * (n + 1)], in_=psk[n]
                            )
                    else:
                        emit_proj(wts[0], qT)
                        emit_proj(wts[1], kT)
                    if h + 1 < NH:
                        next_wts = emit_wdma(h + 1)
                    pcs = {
                        (s, n): p_pool.tile(
                            [P, 4 * n + 4, 512], f16,
                            tag=f"p{n}", name="pch", bufs=4,
                        )
                        for s in range(2)
                        for n in range(2)
                    }
                    if h == 0:
                        backlog = [("v", t) for t in range(8)]
                    else:
                        backlog = [("pv", s, i) for i in range(8) for s in range(2)]
                    if h == NH - 1:
                        # head 7 has no successor iteration: pull its first
                        # four q-tiles' PV units into this head's backlog,
                        # spread through the drain order as their exp chunks
                        # become available
                        for pos, ti in ((8, 3), (12, 2), (16, 1), (20, 0)):
                            backlog[pos:pos] = [("pv7", 0, ti), ("pv7", 1, ti)]
                    sunits = [(n, j) for n in range(2) for j in range(4 * n + 4)]
                    done = 0
                    for idx, (n, j) in enumerate(sunits):
                        score_unit(h, qT, kT, pcs, n, j)
                        while done < len(backlog) and (idx + 1) * len(
                            backlog
                        ) >= (done + 1) * len(sunits):
                            u = backlog[done]
                            done += 1
                            # late head-7 backlog units drain on ACT: exp is
                            # finished there, and the DVE otherwise paces the
                            # PE via psum WAR with no score units left to fill
                            late = h == NH - 1 and done > 16
                            if u[0] == "v":
                                vproj_unit(u[1])
                            elif u[0] == "pv7":
                                pv_unit(h, u[1], u[2], pcs, psY, tail=late)
                            else:
                                pv_unit(h - 1, u[1], u[2], pcs_prev, psY, tail=late)
                    while done < len(backlog):
                        u = backlog[done]
                        done += 1
                        late = h == NH - 1 and done > 16
                        if u[0] == "v":
                            vproj_unit(u[1])
                        elif u[0] == "pv7":
                            pv_unit(h, u[1], u[2], pcs, psY, tail=late)
                        else:
                            pv_unit(h - 1, u[1], u[2], pcs_prev, psY, tail=late)
                    if h == NH - 1:
                        # tiles 3 and 2 finished PV above: run their LN now
                        # so the tail's first transposes never wait on it
                        emit_ln(3)
                        emit_ln(2)
                        # filler matmuls bridge the pool-transition stall so
                        # the HAM activity window never sees the PE idle
                        emit_dummies(1)
                    pcs_prev = pcs
                    if h + 1 < NH:
                        wts = next_wts

            # ---------- tail: PV(7) + LN + transpose pipeline, then c_proj ----------
            with (
                tc.tile_pool(name="smalld", bufs=10) as sd,
                tc.tile_pool(name="ylnT", bufs=8) as ylnT_p,
                tc.tile_pool(name="outp", bufs=3) as out_p,
                tc.tile_pool(name="psY2", bufs=3, space="PSUM") as psY2,
                tc.tile_pool(name="psE", bufs=3, space="PSUM") as psE,
                tc.tile_pool(name="psF", bufs=2, space="PSUM") as psF,
            ):
                ylnT = [ylnT_p.tile([P, T], f16, tag="ylnT", name="ylnT") for _ in range(8)]

                def emit_trans(i):
                    for dh in range(2):
                        pt = psE.tile([P, 512], f16, tag="psE")
                        for w in range(4):
                            dd = 4 * dh + w
                            nc.tensor.transpose(
                                out=pt[:, 128 * w : 128 * (w + 1)],
                                in_=y_tiles[i][:, 128 * dd : 128 * (dd + 1)],
                                identity=ident,
                            )
                        for w in range(4):
                            dd = 4 * dh + w
                            nc.any.tensor_copy(
                                out=ylnT[dd][:, 128 * i : 128 * (i + 1)],
                                in_=pt[:, 128 * w : 128 * (w + 1)],
                            )

                def emit_cproj(i, last=False):
                    # c_proj row-block m = i only needs this q-tile's
                    # transposed columns; the final tile uses 256-col groups
                    # to shorten the end-of-kernel dependency chain
                    osb = out_p.tile([P, C], f16, tag="osb")
                    ngrp, w = (4, 256) if last else (2, 512)
                    for n in range(ngrp):
                        ps = psF.tile([P, w], f32, tag="psF", name="psn")
                        for d in range(8):
                            nc.tensor.matmul(
                                ps,
                                lhsT=ylnT[d][:, 128 * i : 128 * (i + 1)],
                                rhs=wc_sb[d][:, w * n : w * (n + 1)],
                                start=(d == 0),
                                stop=(d == 7),
                            )
                        if last and n % 2 == 1:
                            nc.scalar.activation(
                                out=osb[:, w * n : w * (n + 1)],
                                in_=ps, func=Act.Copy,
                            )
                        else:
                            nc.any.tensor_copy(
                                out=osb[:, w * n : w * (n + 1)], in_=ps
                            )
                        nc.sync.dma_start(
                            out=out_d[128 * i : 128 * (i + 1), w * n : w * (n + 1)],
                            in_=osb[:, w * n : w * (n + 1)],
                        )


                # q-tiles 0-3 finished PV inside the head-7 loop: emit their
                # LN chains first (pure DVE/ACT), then interleave the
                # remaining PV units with their trans+cproj so the PE never
                # waits on the LN latency
                # LN chains are emitted just-in-time (two per iteration) so
                # the DVE queue never delays the tail PV psum drains, which
                # would stall the PE via psum WAR
                ready = [3, 2, 1, 0]
                jit = [1, 0, None, None]
                for k, i in enumerate([7, 6, 5, 4]):
                    for s in range(2):
                        pv_unit(NH - 1, s, i, pcs_prev, psY2, tail=True)
                    # trans+cproj first: its DVE copies must not queue behind
                    # the LN chains in the vector engine's FIFO
                    emit_trans(ready[k])
                    emit_cproj(ready[k])
                    emit_ln(i)
                    if jit[k] is not None:
                        emit_ln(jit[k])
                # software-pipeline the final four tiles: tile i+1's
                # transposes (and their copies) run under tile i's c_proj
                emit_trans(7)
                emit_trans(6)
                emit_cproj(7)
                emit_trans(5)
                emit_cproj(6)
                emit_trans(4)
                emit_cproj(5)
                emit_cproj(4, last=True)

            small_ctx.__exit__(None, None, None)
            p_ctx.__exit__(None, None, None)

    bass._bass_rust.generate_event_semaphores(nc)
    return nc


_NC = None


def _get_program():
    global _NC
    if _NC is None:
        _NC = build_program()
    return _NC


def make_in_maps(inputs):
    """Host-side sharding: per-core input dicts."""
    x = np.ascontiguousarray(np.asarray(inputs["x"], dtype=np.float32))
    Wq1 = np.asarray(inputs["Wq1"], dtype=np.float32)
    Wq2 = np.asarray(inputs["Wq2"], dtype=np.float32)
    Wk1 = np.asarray(inputs["Wk1"], dtype=np.float32)
    Wk2 = np.asarray(inputs["Wk2"], dtype=np.float32)
    Wv = np.asarray(inputs["Wv"], dtype=np.float32)
    Wc = np.asarray(inputs["Wc"], dtype=np.float32)
    lq1 = np.asarray(inputs["lq1"], dtype=np.float32)
    lk1 = np.asarray(inputs["lk1"], dtype=np.float32)
    lq2 = np.asarray(inputs["lq2"], dtype=np.float32)
    lk2 = np.asarray(inputs["lk2"], dtype=np.float32)

    lam1 = np.exp(np.sum(lq1 * lk1, axis=-1))
    lam2 = np.exp(np.sum(lq2 * lk2, axis=-1))
    lam_full = (lam1 - lam2 + LAMBDA_INIT).astype(np.float32)  # (16,)

    in_maps = []
    for core in range(N_CORES):
        b, hg = core // 2, core % 2
        heads = np.arange(NH) + NH * hg  # global head idx
        wq = np.empty((C, C), np.float32)
        wk = np.empty((C, C), np.float32)
        wv = np.empty((C, C), np.float32)
        for h in range(NH):
            H = NH * hg + h
            wq[:, 128 * h : 128 * h + 64] = Wq1[:, HS * H : HS * (H + 1)]
            wq[:, 128 * h + 64 : 128 * (h + 1)] = Wq2[:, HS * H : HS * (H + 1)]
            wk[:, 128 * h : 128 * h + 64] = Wk1[:, HS * H : HS * (H + 1)]
            wk[:, 128 * h + 64 : 128 * (h + 1)] = Wk2[:, HS * H : HS * (H + 1)]
            # center each head's V block: y = att @ V is then exactly
            # row-mean-free, so the LN mean subtraction vanishes (mu == 0)
            blk = Wv[:, 128 * H : 128 * (H + 1)]
            wv[:, 128 * h : 128 * (h + 1)] = blk - blk.mean(axis=1, keepdims=True)
        wc = np.ascontiguousarray(Wc[1024 * hg : 1024 * (hg + 1), :]).astype(np.float16)
        lamneg = np.broadcast_to(
            -lam_full[heads][None, :], (P, NH)
        ).astype(np.float32)
        in_maps.append(
            {
                "xt": np.ascontiguousarray(x[b].T.astype(np.float16)),
                "wq": wq.astype(np.float16),
                "wk": wk.astype(np.float16),
                "wv": wv.astype(np.float16),
                "wc": wc,
                "lamneg": np.ascontiguousarray(lamneg),
            }
        )
    return in_maps


def run(inputs, trace=False, **kw):
    from concourse.bass_utils import run_bass_kernel_spmd

    nc = _get_program()
    in_maps = make_in_maps(inputs)
    res = run_bass_kernel_spmd(
        nc, in_maps, core_ids=list(range(N_CORES)), trace=trace, **kw
    )
    B = 4
    out = np.empty((B, T, C), np.float32)
    for b in range(B):
        out[b] = (
            res.results[2 * b]["out"].astype(np.float32)
            + res.results[2 * b + 1]["out"].astype(np.float32)
        )
    return out, res


def kernel(**inputs) -> np.ndarray:
    out, _ = run(inputs, trace=False)
    return out

